# revision 12
# baseline (speedup 1.0000x reference)
"""Trainium2 Bass kernel for CustomWindowMHA (sparse window+dilated attention).

Sharding: 8 cores = 2 batches x 4 head-groups (4 heads each). Each core
computes QKV projection for its heads, masked attention, and a partial
output projection against its slice of wo's columns; the host sums the 4
partials per batch.

v2 restructure: the dilated mask (j <= i-132, (i-j)%4 == 0) couples only
tokens with equal residue mod 4, so the dilated part is computed in
phase-grouped coordinates (4 independent 512x512 causal-offset attentions
per head) instead of densely: 4x less score/exp work in the dilated
region and masks shrink to two constant 128x128 deltas. The 128-wide
window band stays in natural token order (2 j-tiles per q-tile, T0/T1
triangle masks). Both parts share one softmax: window PV accumulates
poT[65, 512] per (head, chunk) in PSUM, dilated PV accumulates
poTd[65, 512] per (head, phase) which is flushed to SBUF (pdacc) and
merged at normalize time via an interleaving access pattern.

Layouts (all matmuls bf16, fp32 PSUM):
  - Q^T/K^T [dh, S] natural via transposed projection; Q^T additionally
    staged phase-major (qphase) straight from the projection PSUM.
  - Phase-sliced K^T / x columns are read with stride-4 access patterns.
  - V token-major with fused ones-column ([V|1]), projected twice: 16
    natural tiles + 16 phase tiles, so both PV parts see their j-order
    on partitions; the PV matmul yields out^T AND the softmax
    denominator row in one accumulation.
  - Head pairs co-run their K=64 score matmuls in disjoint PE row halves.
  - Projections / V-projections / wo are woven between attention units so
    the PE never idles while ACT computes exp (keeps HAM un-throttled).
"""

import sys

sys.path.insert(0, "/opt/trn_rl_repo")

import numpy as np
import ml_dtypes

import concourse.bass as bass
import concourse.mybir as mybir
import concourse.tile as tile
from concourse.vector_clock import ScopedClock
from concourse.bass_utils import run_bass_kernel_spmd

BF16 = mybir.dt.bfloat16
F32 = mybir.dt.float32

B, S, D = 2, 2048, 1024
H, DH = 16, 64
WINDOW, DILATION = 128, 4
P = 128
NT = S // P          # 16 token tiles
KT = D // P          # 8 contraction tiles over D
HPC = 4              # heads per core
QC = 512             # q-chunk width
NQC = S // QC        # 4 q-chunks
NPH = 4              # phases (token residue mod 4)
SP = S // NPH        # 512 tokens per phase
W3 = 3 * HPC * DH    # 768 qkvt columns per k-tile


class _TileContext(tile.TileContext):
    """Kernel-tail Drain gets one wait per live proc, but this walrus build
    allows only a single sync wait on SP Drain — split across drains."""

    def _drain_and_barrier(self, tick_clock, wait_clock):
        drain_inst = self.nc.sync.drain()
        wait_clock.add_sem_waits(
            drain_inst.ins, ScopedClock({None: tick_clock.global_clock})
        )
        si = drain_inst.ins.sync_info
        if si is not None and len(si.on_wait) > 1:
            waits = list(si.on_wait)
            si.on_wait[:] = waits[:1]
            for w in waits[1:]:
                d2 = self.nc.sync.drain()
                si2 = d2.ins.sync_info
                if si2 is None:
                    d2.ins.sync_info = mybir.SyncInfo(on_wait=[w], on_update=[])
                else:
                    si2.on_wait[:] = [w]

        self.nc.all_engine_barrier()
        assert self.sems is not None
        popped = self.nc._tile_sem_poison_stack.pop()
        assert popped is self._sem_poison
        self.nc.clear_and_free_semaphores(list(self.sems.allocated().values()))
        self.nc.all_engine_barrier()


def _split_sync_waits(nc):
    """This walrus build allows only one sync-wait slot on several ISA
    structs. Rewrite the scheduled BIR so every instruction carries at most
    one wait: extra waits move onto same-engine NoOps inserted just before
    (same engine queue => executes in order => semantics preserved)."""
    cnt = 0
    for fn in nc.m.functions:
        for blk in fn.blocks:
            new_insts = []
            for inst in blk.instructions:
                si = inst.sync_info
                if si is not None and si.on_wait and len(si.on_wait) > 1:
                    waits = list(si.on_wait)
                    si.on_wait[:] = waits[-1:]
                    for w in waits[:-1]:
                        cnt += 1
                        nop = mybir.InstNoOp(
                            name=f"waitsplit-{cnt}",
                            engine=inst.engine,
                            ins=[],
                            outs=[],
                            sync_info=mybir.SyncInfo(on_wait=[w], on_update=[]),
                        )
                        new_insts.append(nop)
                new_insts.append(inst)
            blk.instructions[:] = new_insts
    return cnt


def _mask_table() -> np.ndarray:
    """[128, 1280] bf16 mask table:
      cols    0:512  maskwA = [T1 T0 T1 T0]  (window set-A composite)
      cols  512:1024 maskwB = [T0 T1 T0 T1]  (window set-B composite)
      cols 1024:1280 maskd  = [Bd | Ad]      (dilated diag / super-diag)
    where, in ST[j, q] orientation (j = partition, q = free):
      T0[sj, sq] = sq >= sj         (window tile (jt, jt))
      T1[sj, sq] = sq <  sj         (window tile (jt, jt+1))
      Bd[skj, sqi] = sqi - skj >= 33   (dilated phase tile qc == kj)
      Ad[skj, sqi] = sqi - skj >= -95  (dilated phase tile qc == kj+1)
    """
    sj = np.arange(P)[:, None]
    sq = np.arange(P)[None, :]
    t0 = (sq >= sj).astype(ml_dtypes.bfloat16)
    t1 = (sq < sj).astype(ml_dtypes.bfloat16)
    bd = ((sq - sj) >= 33).astype(ml_dtypes.bfloat16)
    ad = ((sq - sj) >= -95).astype(ml_dtypes.bfloat16)
    out = np.zeros((P, 10 * P), dtype=ml_dtypes.bfloat16)
    for k, m in enumerate([t1, t0, t1, t0, t0, t1, t0, t1, bd, ad]):
        out[:, k * P : (k + 1) * P] = m
    return out


def _build_program(repeat: int = 1):
    nc = bass.Bass("TRN2", target_bir_lowering=False, debug=False)

    xt_d = nc.declare_dram_parameter("xt", [D, S], BF16, isOutput=False)
    qkvt_d = nc.declare_dram_parameter("qkvt", [D, W3], BF16, isOutput=False)
    wot_d = nc.declare_dram_parameter("wot", [HPC * DH, D], BF16, isOutput=False)
    mask_d = nc.declare_dram_parameter("mask", [P, 10 * P], BF16, isOutput=False)
    y_d = nc.declare_dram_parameter("y", [S, D], F32, isOutput=True)

    with _TileContext(nc) as tc:
        with (
            tc.tile_pool(name="const", bufs=1) as cpool,
            tc.tile_pool(name="work", bufs=2) as wpool,
            tc.tile_pool(name="psum", bufs=2, space="PSUM") as pspool,
        ):
            # ---- persistent SBUF tensors ----
            # per-kt tiles so the projection can start as soon as the first
            # k-slice of the input DMA lands
            xt_sb = [cpool.tile([P, S], BF16, tag=f"xt{kt}", name=f"xt{kt}") for kt in range(KT)]
            qkvt_sb = [cpool.tile([P, W3], BF16, tag=f"qkvt{kt}", name=f"qkvt{kt}") for kt in range(KT)]
            mask_sb = cpool.tile([P, 10 * P], BF16, tag="mask")
            wot_sb = cpool.tile([P, 2 * D], BF16, tag="wot")
            # natural-order Q^T/K^T per 512-token chunk:
            # cols [mt*QC + t]: mt 0,1 = Q channels 0:128/128:256 (head pairs
            # 0,1); mt 2,3 = K channels. partition = channel within pair.
            qkt_cc = [cpool.tile([P, 4 * QC], BF16, tag=f"qkt{cc}", name=f"qkt{cc}") for cc in range(NQC)]
            # phase-major Q^T staging per head pair: [128 chans, r, qi]
            qphase = [cpool.tile([P, NPH, SP], BF16, tag=f"qph{hp}", name=f"qph{hp}") for hp in range(2)]
            # V (+ones col) natural tiles and phase tiles
            vaug_n = [
                cpool.tile([P, HPC, DH + 1], BF16, tag=f"vn{nt}", name=f"vn{nt}") for nt in range(NT)
            ]
            vaug_p = [
                [cpool.tile([P, HPC, DH + 1], BF16, tag=f"vp{r}_{kj}", name=f"vp{r}_{kj}") for kj in range(NPH)]
                for r in range(NPH)
            ]
            # dilated accumulators flushed from PSUM: per head [65, r, qi] f32
            pdacc = [
                cpool.tile([DH + 1, NPH, SP], F32, tag=f"pd{h}", name=f"pd{h}")
                for h in range(HPC)
            ]
            outt_sb = cpool.tile([P, 2 * S], BF16, tag="outt")  # out^T, ct-major
            ones1_sb = cpool.tile([1, 64], mybir.dt.float16, tag="ones1")

            for nt in range(NT):
                nc.gpsimd.memset(vaug_n[nt][:], 1.0)
            for r in range(NPH):
                for kj in range(NPH):
                    nc.gpsimd.memset(vaug_p[r][kj][:], 1.0)
            nc.gpsimd.memset(ones1_sb[:], 1.0)

            for _rep in range(repeat):
                # ---- input DMA (per-kt tiles => fine-grained deps) ----
                for kt in range(KT):
                    nc.sync.dma_start(
                        out=xt_sb[kt][:], in_=xt_d[kt * P : (kt + 1) * P, :]
                    )
                    nc.sync.dma_start(
                        out=qkvt_sb[kt][:], in_=qkvt_d[kt * P : (kt + 1) * P, :]
                    )
                nc.sync.dma_start(out=mask_sb[:], in_=mask_d[:])
                for ct in range(2):
                    nc.sync.dma_start(
                        out=wot_sb[:, ct * D : (ct + 1) * D],
                        in_=wot_d[ct * P : (ct + 1) * P, :],
                    )

                # ---- QT / KT projection (transposed) ----
                # mt 0,1 = Q channels 0:128 / 128:256; mt 2,3 = K channels.
                for mt in range(4):
                    coloff = (0, 128, 256, 384)[mt]
                    for cc in range(NQC):
                        ps = pspool.tile([P, QC], F32, tag="ps512")
                        for kt in range(KT):
                            nc.tensor.matmul(
                                ps[:],
                                lhsT=qkvt_sb[kt][:, coloff : coloff + P],
                                rhs=xt_sb[kt][:, cc * QC : (cc + 1) * QC],
                                start=(kt == 0),
                                stop=(kt == KT - 1),
                            )
                        nc.vector.tensor_copy(
                            qkt_cc[cc][:, mt * QC : (mt + 1) * QC], ps[:]
                        )
                        if mt < 2:
                            # phase-major restage: col 4c+f -> [f, c]
                            nc.vector.tensor_copy(
                                qphase[mt][:, :, cc * P : (cc + 1) * P],
                                ps[:].rearrange("p (c f) -> p f c", f=NPH),
                            )

                # ---- V projection macros (woven into attention below) ----
                def _vproj_nat(nt):
                    def _go():
                        ps = pspool.tile([P, HPC * DH], F32, tag="ps512")
                        for kt in range(KT):
                            nc.tensor.matmul(
                                ps[:],
                                lhsT=xt_sb[kt][:, nt * P : (nt + 1) * P],
                                rhs=qkvt_sb[kt][:, 512:768],
                                start=(kt == 0),
                                stop=(kt == KT - 1),
                            )
                        nc.vector.tensor_copy(
                            vaug_n[nt][:, :, 0:DH],
                            ps[:].rearrange("p (h d) -> p h d", h=HPC),
                        )

                    return _go

                def _vproj_ph(r, kj):
                    def _go():
                        ps = pspool.tile([P, HPC * DH], F32, tag="ps512")
                        for kt in range(KT):
                            nc.tensor.matmul(
                                ps[:],
                                lhsT=xt_sb[kt][:, kj * QC : (kj + 1) * QC].rearrange(
                                    "p (c f) -> p f c", f=NPH
                                )[:, r, :],
                                rhs=qkvt_sb[kt][:, 512:768],
                                start=(kt == 0),
                                stop=(kt == KT - 1),
                            )
                        nc.vector.tensor_copy(
                            vaug_p[r][kj][:, :, 0:DH],
                            ps[:].rearrange("p (h d) -> p h d", h=HPC),
                        )

                    return _go

                # filler queue: PE-heavy macros woven between attention units
                fillers = []
                for r in range(NPH):
                    for kj in range(NPH):
                        fillers.append(_vproj_ph(r, kj))
                for nt in range(NT):
                    fillers.append(_vproj_nat(nt))

                pending = []

                def _flush_pending():
                    while pending:
                        pending.pop(0)()

                # ---------- normalize + wo (same machinery as v1) ----------
                def _normalize_p1(h, qc, poT):
                    pb = 64 * (h % 2)
                    qoff = (h // 2) * S
                    # [65, f, c] -> iterate c outer, f inner: natural 4c+f order
                    pdv = pdacc[h][:, :, qc * P : (qc + 1) * P].transpose([0, 2, 1])
                    po_sb = wpool.tile([64, QC], F32, tag="posb", bufs=4)
                    nc.vector.scalar_tensor_tensor(
                        out=po_sb[:],
                        in0=poT[0:64, :],
                        scalar=1.0,
                        in1=pdv[0:64, :],
                        op0=mybir.AluOpType.mult,
                        op1=mybir.AluOpType.add,
                    )
                    lrow = wpool.tile([1, QC], F32, tag="lrow", bufs=4)
                    nc.vector.scalar_tensor_tensor(
                        out=lrow[:],
                        in0=poT[64:65, :],
                        scalar=1.0,
                        in1=pdv[64:65, :],
                        op0=mybir.AluOpType.mult,
                        op1=mybir.AluOpType.add,
                    )
                    r4 = wpool.tile([P, 4], F32, tag="r4", bufs=4)
                    nc.sync.dma_start(
                        out=r4[:],
                        in_=lrow[0:1, :].rearrange("a (p c) -> a p c", p=P),
                    )
                    i4h = wpool.tile([P, 4], mybir.dt.float16, tag="i4h", bufs=4)
                    with nc.allow_low_precision("softmax 1/L in fp16"):
                        nc.vector.reciprocal(i4h[:], r4[:])
                    invl16 = wpool.tile([1, QC], mybir.dt.float16, tag="invl16", bufs=4)
                    nc.sync.dma_start(
                        out=invl16[0:1, :].rearrange("a (p c) -> a p c", p=P),
                        in_=i4h[:],
                    )

                    def _p2():
                        ib = pspool.tile([64, QC], F32, tag="ps512", bufs=2)
                        nc.tensor.matmul(
                            ib[:], lhsT=ones1_sb[:], rhs=invl16[:], start=True, stop=True
                        )
                        nc.vector.scalar_tensor_tensor(
                            out=outt_sb[
                                pb : pb + 64, qoff + qc * QC : qoff + (qc + 1) * QC
                            ],
                            in0=po_sb[:],
                            scalar=1.0,
                            in1=ib[:],
                            op0=mybir.AluOpType.mult,
                            op1=mybir.AluOpType.mult,
                        )

                    pending.append(_p2)

                def _emit_wo(qc):
                    def _go():
                        for qt in range(4 * qc, 4 * qc + 4):
                            ysb = wpool.tile([P, D], F32, tag="ysb", bufs=2)
                            for oc in range(2):
                                yps = pspool.tile([P, QC], F32, tag="ps512")
                                for ct in range(2):
                                    nc.tensor.matmul(
                                        yps[:],
                                        lhsT=outt_sb[
                                            :, ct * S + qt * P : ct * S + (qt + 1) * P
                                        ],
                                        rhs=wot_sb[
                                            :, ct * D + oc * QC : ct * D + (oc + 1) * QC
                                        ],
                                        start=(ct == 0),
                                        stop=(ct == 1),
                                    )
                                if oc == 0:
                                    nc.scalar.copy(
                                        ysb[:, oc * QC : (oc + 1) * QC], yps[:]
                                    )
                                else:
                                    nc.vector.tensor_copy(
                                        ysb[:, oc * QC : (oc + 1) * QC], yps[:]
                                    )
                            nc.sync.dma_start(
                                out=y_d[qt * P : (qt + 1) * P, :], in_=ysb[:]
                            )

                    pending.append(_go)

                # ---------- dilated attention (phase-grouped) ----------
                # per head pair hp, per phase r, per phase-j-tile kj:
                #   ST[kj*128.., qi in 128kj:512] co-run for the pair,
                #   exp, small masks (diag Bd / super Ad), PV into poTd.
                def _run_dilated(hp):
                    ps_t, e_t = {}, {}
                    poTd = [None, None]

                    def _issue_st(u):
                        r, kj = u
                        n = (NPH - kj) * P
                        pair = []
                        for i, pb in enumerate((0, 64)):
                            ps = pspool.tile([P, n], F32, tag="st", bufs=4)
                            nc.tensor.matmul(
                                ps[:],
                                lhsT=qkt_cc[kj][
                                    pb : pb + 64, (2 + hp) * QC : (3 + hp) * QC
                                ].rearrange("p (c f) -> p f c", f=NPH)[:, r, :],
                                rhs=qphase[hp][pb : pb + 64, r, kj * P : SP],
                                start=True,
                                stop=True,
                            )
                            pair.append(ps)
                        ps_t[u] = pair

                    def _issue_exp(u):
                        r, kj = u
                        n = (NPH - kj) * P
                        mw = min(2 * P, n)  # masked width: Bd then Ad
                        pair = []
                        for i in range(2):
                            e = wpool.tile([P, n], BF16, tag="e", bufs=8)
                            nc.scalar.activation(
                                e[:],
                                ps_t[u][i][:],
                                mybir.ActivationFunctionType.Exp,
                                scale=0.125,
                            )
                            eng = nc.gpsimd if kj >= 2 else nc.vector
                            eng.tensor_mul(
                                e[:, 0:mw], e[:, 0:mw], mask_sb[:, 8 * P : 8 * P + mw]
                            )
                            pair.append(e)
                        del ps_t[u]
                        e_t[u] = pair

                    def _issue_pv(u):
                        r, kj = u
                        if kj == 0:
                            poTd[0] = pspool.tile([P, SP], F32, tag="pot", name="potd0")
                            poTd[1] = pspool.tile([P, SP], F32, tag="pot", name="potd1")
                        for i in range(2):
                            nc.tensor.matmul(
                                poTd[i][0:65, kj * P : SP],
                                lhsT=vaug_p[r][kj][:, 2 * hp + i, :],
                                rhs=e_t[u][i][:],
                                start=(kj == 0),
                                stop=(kj == NPH - 1),
                            )
                        del e_t[u]
                        if kj == NPH - 1:
                            for i in range(2):
                                nc.scalar.copy(
                                    pdacc[2 * hp + i][:, r, :], poTd[i][0:65, :]
                                )

                    units = [(r, kj) for r in range(NPH) for kj in range(NPH)]
                    nu = len(units)
                    for step in range(nu + 2):
                        if step < nu:
                            _issue_st(units[step])
                            if fillers:
                                fillers.pop(0)()
                        if 0 <= step - 1 < nu:
                            _issue_exp(units[step - 1])
                        if 0 <= step - 2 < nu:
                            _issue_pv(units[step - 2])

                _run_dilated(0)
                _run_dilated(1)

                # ---------- window attention + normalize + wo ----------
                # per (qc, hp): two packed score sets:
                #   set A: jts {4qc-1, 4qc+1, 4qc+3} -> psum cols [0:128,
                #          128:384, 384:512] (qc=0 drops jt=-1)
                #   set B: jts {4qc, 4qc+2}          -> psum cols [0:256,
                #          256:512]
                # masks: A -> maskwA [T1 T0 T1 T0], B -> maskwB [T0 T1 T0 T1]
                # PV splits per 128-col block with T0 (start=True) issued
                # before T1 (start=False) for each block.
                def _win_st(qc, hp):
                    plans = {
                        "A": [(4 * qc - 1, 0, P), (4 * qc + 1, P, 3 * P),
                              (4 * qc + 3, 3 * P, 4 * P)],
                        "B": [(4 * qc, 0, 2 * P), (4 * qc + 2, 2 * P, 4 * P)],
                    }
                    sets = {}
                    for sk, plan in plans.items():
                        pair = []
                        for i, pb in enumerate((0, 64)):
                            ps = pspool.tile([P, QC], F32, tag="st", bufs=4)
                            for jt, c0, c1 in plan:
                                if jt < 0:
                                    continue
                                nc.tensor.matmul(
                                    ps[:, c0:c1],
                                    lhsT=qkt_cc[jt // 4][
                                        pb : pb + 64,
                                        (2 + hp) * QC
                                        + (jt % 4) * P : (2 + hp) * QC
                                        + (jt % 4 + 1) * P,
                                    ],
                                    rhs=qkt_cc[qc][
                                        pb : pb + 64, hp * QC + c0 : hp * QC + c1
                                    ],
                                    start=True,
                                    stop=True,
                                )
                            pair.append(ps)
                        sets[sk] = pair
                    return sets

                def _win_expmask(qc, hp, sets):
                    a0 = P if qc == 0 else 0
                    es = {}
                    for sk, moff in (("A", 0), ("B", 4 * P)):
                        c0 = a0 if sk == "A" else 0
                        pair = []
                        for i in range(2):
                            e = wpool.tile([P, QC], BF16, tag="e", bufs=8)
                            nc.scalar.activation(
                                e[:, c0:],
                                sets[sk][i][:, c0:],
                                mybir.ActivationFunctionType.Exp,
                                scale=0.125,
                            )
                            nc.vector.tensor_mul(
                                e[:, c0:],
                                e[:, c0:],
                                mask_sb[:, moff + c0 : moff + QC],
                            )
                            pair.append(e)
                        es[sk] = pair
                    return es

                def _win_pv(qc, hp, es, poTs):
                    # One accumulation group per poT bank: the first issued
                    # matmul start=True pending-zeroes the whole bank row, so
                    # later start=False writes overwrite pending bytes and
                    # accumulate already-written ones. Each jt's T0+T1 blocks
                    # are contiguous in its set's e-tile, so a single N=256
                    # matmul covers both. (jt, set, e-col0, width)
                    plan = [
                        (4 * qc, "B", 0, 2 * P),
                        (4 * qc + 2, "B", 2 * P, 2 * P),
                        (4 * qc + 1, "A", P, 2 * P),
                        (4 * qc + 3, "A", 3 * P, P),
                        (4 * qc - 1, "A", 0, P),
                    ]
                    plan = [p for p in plan if p[0] >= 0]
                    for n_, (jt, sk, ec, w) in enumerate(plan):
                        for i in range(2):
                            nc.tensor.matmul(
                                poTs[i][0:65, ec : ec + w],
                                lhsT=vaug_n[jt][:, 2 * hp + i, :],
                                rhs=es[sk][i][:, ec : ec + w],
                                start=(n_ == 0),
                                stop=(n_ == len(plan) - 1),
                            )

                units = [(qc, hp) for qc in range(NQC) for hp in range(2)]
                nu = len(units)
                stq, eq = {}, {}
                poT_u = {}
                for step in range(nu + 2):
                    if step < nu:
                        stq[units[step]] = _win_st(*units[step])
                        if fillers:
                            fillers.pop(0)()
                    if 0 <= step - 1 < nu:
                        u = units[step - 1]
                        eq[u] = _win_expmask(*u, stq.pop(u))
                    if 0 <= step - 2 < nu:
                        u = units[step - 2]
                        qc, hp = u
                        _flush_pending()
                        poTs = [
                            pspool.tile([P, QC], F32, tag="pot", name="potw0"),
                            pspool.tile([P, QC], F32, tag="pot", name="potw1"),
                        ]
                        _win_pv(qc, hp, eq.pop(u), poTs)
                        _normalize_p1(2 * hp, qc, poTs[0])
                        _normalize_p1(2 * hp + 1, qc, poTs[1])
                        if hp == 1:
                            _emit_wo(qc)
                while fillers:
                    fillers.pop(0)()
                _flush_pending()

    _split_sync_waits(nc)
    return nc


_PROGRAMS = {}


def _program(repeat: int = 1):
    if repeat not in _PROGRAMS:
        _PROGRAMS[repeat] = _build_program(repeat)
    return _PROGRAMS[repeat]


def _prep_inputs(x, qkv, wo):
    """Per-core host-side slicing/transposition/casting."""
    mask = _mask_table()
    in_maps = []
    for c in range(8):
        b, hg = c // 4, c % 4
        h0 = HPC * hg
        rows = np.r_[
            h0 * DH : h0 * DH + HPC * DH,
            D + h0 * DH : D + h0 * DH + HPC * DH,
            2 * D + h0 * DH : 2 * D + h0 * DH + HPC * DH,
        ]
        qkvt = np.ascontiguousarray(qkv[rows].T).astype(ml_dtypes.bfloat16)
        xt = np.ascontiguousarray(x[b].T).astype(ml_dtypes.bfloat16)
        wot = np.ascontiguousarray(
            wo[:, h0 * DH : h0 * DH + HPC * DH].T
        ).astype(ml_dtypes.bfloat16)
        in_maps.append({"xt": xt, "qkvt": qkvt, "wot": wot, "mask": mask})
    return in_maps


def kernel(x, qkv, wo, _trace=False, _trace_kwargs=None):
    x = np.asarray(x, dtype=np.float32)
    qkv = np.asarray(qkv, dtype=np.float32)
    wo = np.asarray(wo, dtype=np.float32)

    nc = _program()
    in_maps = _prep_inputs(x, qkv, wo)
    res = run_bass_kernel_spmd(
        nc, in_maps, list(range(8)), trace=_trace, **(_trace_kwargs or {})
    )
    kernel.last_result = res

    y = np.zeros((B, S, D), dtype=np.float32)
    for c in range(8):
        y[c // 4] += res.results[c]["y"]
    return y
